# revision 1
# baseline (speedup 1.0000x reference)
"""Trainium2 Bass kernel for DynamicSparseAttention.

Reference computation (per batch b, head h):
    scores  = Q @ K^T                      [L, S]
    dense   = softmax(scores, axis=-1)
    routing = dense ** 5
    combined = (routing + dense) * 0.5
    sparse  = combined / sum(combined, -1, keepdims=True)
    out     = sparse @ V                   [L, D]

Math: with p = exp(s - m), Z = sum_s p (any per-row shift m),
    out = (P5 @ V + Z^4 * (P @ V)) / (W + Z^5),  W = sum_s p^5.
Ones-columns appended to V make the matmul accumulators carry the row sums.

Numerical strategy (two rounds, unconditionally stable):
  Round A uses a loose per-row shift m1 ~ sqrt(2 ln S)|q_l| + 25 (anything
  within ~+-80 of the true row max works; the bf16-rounded applied value is
  what matters and is self-consistent).  Its ones-column yields Z_A, i.e.
  the per-row logsumexp m2 = m1 + ln Z_A.  Round B recomputes scores
  shifted by m2 (so sum exp(s - m2) ~= 1) and accumulates A = P5 @ [V|1]
  with p5 = exp(5(s - m2)) in [e^-38, 1] — no overflow/underflow possible.
    out = (A + B/Z_A)[:, :D] / (A + B/Z_A)[:, D].

All matmuls run in bf16 (the fp32/f32r moving-operand path streams at half
rate).  fp32 precision for the scores is recovered with a hi/lo split:
  s = khi.(qhi+qlo) + klo.qhi - shift      (error |klo.qlo| ~ 3e-5)
as two accumulating bf16 matmuls per chunk:
  mm1: lhsT = [khi;khi] (K=128)   rhs = [qhi;qlo]
  mm2: lhsT = [klo|1(|1|1)]       rhs = [qhi|-m1(|-lnZhi|-lnZlo)]
Round A contracts mm2 channels 0..64 (shift m1); round B channels 0..66,
adding the two -lnZ rows (hi+lo bf16 decomposition, so the applied round-B
shift equals m1 + lnZ to ~2e-4 — the epilogue's B/Z_A rescale then matches
the applied shifts to ~1e-3 on the dense/routing balance).

The -lnZ rows are computed per l-half without any cross-partition moves:
V carries THREE ones-columns (cols 64..66), so the accumulator holds Z at
partitions 64, 65 AND 66; partition-local ACT Ln / DVE ops write the qtB
shift rows 65 and 66 in place.

Layout: scores are computed transposed, [s partitions, l free], so the
exp() outputs feed the P@V' matmuls directly (contraction over s on
partitions, V' stationary).

Sharding: B*H = 32 (b,h) pairs, 4 per core across 8 cores, no cross-core
communication.  kernel() takes full inputs and returns the full output.
"""

import os
import sys
import numpy as np

for _p in ("/opt/trn_rl_repo",):
    if os.path.isdir(_p) and _p not in sys.path:
        sys.path.insert(0, _p)

from contextlib import ExitStack

import json as _json

import ml_dtypes

import concourse.bass as bass
import concourse.mybir as mybir
import concourse.tile as tile
import concourse.bass2jax as _bass2jax
import concourse.bass_utils as _bass_utils
from concourse.bass_utils import run_bass_kernel_spmd
from concourse.masks import make_identity

# ---------------------------------------------------------------------------
# Workaround: this container's walrus build rejects instructions carrying
# more than one sync wait ("Too many sync wait commands", CoreV3GenImpl
# setupSyncWait<...>).  Tile's scheduler freely attaches 2-3 waits per
# instruction (and ~27 on the tail drain).  Rewrite the BIR JSON before
# compilation: excess waits are hoisted onto freshly inserted same-engine
# NoOp instructions placed immediately before the instruction, one wait
# each.  Semantics are unchanged (waits are conjunctive >= conditions and
# engine program order is preserved).
# ---------------------------------------------------------------------------

_MAX_WAITS = 1


def _split_waits_in_bir(bir_json: bytes) -> bytes:
    bir = _json.loads(bir_json)
    n_new = [0]

    def fix_block(bb):
        out = []
        for inst in bb["instructions"]:
            si = inst.get("sync_info") or {}
            waits = si.get("on_wait") or []
            if len(waits) > _MAX_WAITS:
                excess, keep = waits[:-_MAX_WAITS], waits[-_MAX_WAITS:]
                for w in excess:
                    n_new[0] += 1
                    out.append({
                        "debug": inst.get("debug", 0),
                        "engine": inst["engine"],
                        "ins": [],
                        "name": "I-wsplit-%d" % n_new[0],
                        "opcode": "NoOp",
                        "outs": [],
                        "sync_info": {"on_update": [], "on_wait": [w]},
                    })
                si["on_wait"] = keep
            out.append(inst)
        bb["instructions"] = out

    for fn in bir["functions"]:
        for bb in fn["blocks"]:
            fix_block(bb)
    return _json.dumps(bir).encode()


_orig_compile_bir_kernel = _bass_utils.compile_bir_kernel


def _patched_compile_bir_kernel(bir_json, tmpdir, neff_name="file.neff"):
    return _orig_compile_bir_kernel(
        _split_waits_in_bir(bir_json), tmpdir, neff_name=neff_name
    )


_bass_utils.compile_bir_kernel = _patched_compile_bir_kernel
_bass2jax.compile_bir_kernel = _patched_compile_bir_kernel

B, L, S, H, E, D = 2, 2048, 2048, 16, 64, 64
NCORES = 8
NP = (B * H) // NCORES  # pairs per core = 4
EB = E + 3  # mm2 channels: 64 klo/qhi + 2 lnZ rows (64,65) + m1 row (66)
DV = D + 2  # v columns: 64 data + 2 ones columns (Z at partitions 64,65)
LT = L // 128
ST = S // 128
LHALF = 1024  # l columns per accumulation pass (PSUM capacity)
NCH = 2  # 512-wide matmul chunks per l-half
NLH = L // LHALF
FACTOR = 5.0

F32 = mybir.dt.float32
BF16 = mybir.dt.bfloat16
EXP = mybir.ActivationFunctionType.Exp
LN = mybir.ActivationFunctionType.Ln
COPY = mybir.ActivationFunctionType.Copy

M_COEF = float(np.sqrt(2.0 * np.log(S)))
M_MARGIN = 25.0


def _emit(ctx: ExitStack, tc: tile.TileContext, qa, qb, ka, kb, va, outp):
    nc = tc.nc

    const = ctx.enter_context(tc.tile_pool(name="const", bufs=1))
    nat = ctx.enter_context(tc.tile_pool(name="nat", bufs=4))
    big = ctx.enter_context(tc.tile_pool(name="big", bufs=2))
    vpool = ctx.enter_context(tc.tile_pool(name="vp", bufs=2))
    ppool = ctx.enter_context(tc.tile_pool(name="pp", bufs=4))
    eppool = ctx.enter_context(tc.tile_pool(name="ep", bufs=2))
    opool = ctx.enter_context(tc.tile_pool(name="op", bufs=4))
    zpool = ctx.enter_context(tc.tile_pool(name="zp", bufs=2))

    ps_sc = ctx.enter_context(tc.tile_pool(name="ps_sc", bufs=2, space="PSUM"))
    ps_acc = ctx.enter_context(tc.tile_pool(name="ps_acc", bufs=2, space="PSUM"))

    identb = const.tile([128, 128], BF16)
    make_identity(nc, identb)
    ident65 = const.tile([D + 1, D + 1], F32)
    make_identity(nc, ident65)
    # row-select masks for the hi/lo -lnZ write window [64:66]
    msk = const.tile([EB, 1], F32)   # 1 at row 64, 0 at row 65
    imsk = const.tile([EB, 1], F32)  # 0 at row 64, 1 at row 65
    nc.vector.memset(msk[E:E + 2, :], 0.0)
    nc.vector.memset(msk[E:E + 1, :], 1.0)
    nc.vector.memset(imsk[E:E + 2, :], 1.0)
    nc.vector.memset(imsk[E:E + 1, :], 0.0)

    for bh in range(NP):
        # ---- setup: load/transpose Q,K into [channels, L] bf16 tiles ----
        qtA = big.tile([128, L], BF16, tag="qtA")   # [qhi; qlo]
        qtB = big.tile([EB, L], BF16, tag="qtB")    # [qhi | -m1 | lnZ rows]
        ktA = big.tile([128, S], BF16, tag="ktA")   # [khi; khi]
        ktB = big.tile([EB, S], BF16, tag="ktB")    # [klo | 1 | 1 | 1]
        for t in range(LT):
            qnA = nat.tile([128, 128], BF16, tag="natA", name="qnA")
            nc.sync.dma_start(out=qnA, in_=qa[bh, t * 128:(t + 1) * 128, :])
            tpA = ps_sc.tile([128, 128], BF16, tag="sc", name="tpA")
            nc.tensor.transpose(tpA, qnA, identb)
            nc.vector.tensor_copy(qtA[:, t * 128:(t + 1) * 128], tpA)
            qnB = nat.tile([128, EB], BF16, tag="natB", name="qnB")
            nc.sync.dma_start(out=qnB, in_=qb[bh, t * 128:(t + 1) * 128, :])
            tpB = ps_sc.tile([EB, 128], BF16, tag="sc", name="tpB")
            nc.tensor.transpose(tpB, qnB, identb)
            nc.vector.tensor_copy(qtB[:, t * 128:(t + 1) * 128], tpB)
        for t in range(ST):
            knA = nat.tile([128, 128], BF16, tag="natA", name="knA")
            nc.sync.dma_start(out=knA, in_=ka[bh, t * 128:(t + 1) * 128, :])
            tpKA = ps_sc.tile([128, 128], BF16, tag="sc", name="tpKA")
            nc.tensor.transpose(tpKA, knA, identb)
            nc.vector.tensor_copy(ktA[:, t * 128:(t + 1) * 128], tpKA)
            knB = nat.tile([128, EB], BF16, tag="natB", name="knB")
            nc.sync.dma_start(out=knB, in_=kb[bh, t * 128:(t + 1) * 128, :])
            tpKB = ps_sc.tile([EB, 128], BF16, tag="sc", name="tpKB")
            nc.tensor.transpose(tpKB, knB, identb)
            nc.vector.tensor_copy(ktB[:, t * 128:(t + 1) * 128], tpKB)

        vts = []
        for t in range(ST):
            vt = vpool.tile([128, DV], BF16, tag=f"v{t}", name=f"vt{t}")
            nc.sync.dma_start(out=vt, in_=va[bh, t * 128:(t + 1) * 128, :])
            vts.append(vt)

        # ---- main: round A for both l-halves, then round B ----
        b_sbs, a_sbs = {}, {}
        for lh in range(NLH):
            l0 = lh * LHALF
            accb = ps_acc.tile([DV, LHALF], F32, tag="acc", name="accb")
            # round A: p = exp(s - m1); accumulate B = P @ [V|1s]
            for st in range(ST):
                sb = slice(st * 128, (st + 1) * 128)
                sc = ps_sc.tile([128, LHALF], F32, tag="sc", name="scA")
                for c in range(NCH):
                    cs = slice(c * 512, (c + 1) * 512)
                    gs = slice(l0 + c * 512, l0 + (c + 1) * 512)
                    nc.tensor.matmul(sc[:, cs], lhsT=ktA[:, sb],
                                     rhs=qtA[:, gs], start=True, stop=False)
                    nc.tensor.matmul(sc[:, cs], lhsT=ktB[:, sb],
                                     rhs=qtB[:, gs], start=False, stop=True)
                p = ppool.tile([128, LHALF], BF16, tag="p", name="p")
                nc.scalar.activation(p, sc, EXP, bias=0.0, scale=1.0)
                for c in range(NCH):
                    cs = slice(c * 512, (c + 1) * 512)
                    nc.tensor.matmul(accb[:, cs], lhsT=vts[st], rhs=p[:, cs],
                                     start=(st == 0), stop=(st == ST - 1))

            # mid: qtB rows 64/65 <- -lnZhi, -lnZlo.  All ops work on the
            # 32-aligned partition window [64:66]; Z is replicated at
            # accumulator partitions 64 and 65 (two ones-columns in V'),
            # and per-partition masks select the hi vs lo row.
            w = slice(E, E + 2)
            zr = zpool.tile([EB, LHALF], F32, tag="zr", name="zr")
            zh = zpool.tile([EB, LHALF], BF16, tag="zh", name="zh")
            zs = zpool.tile([EB, LHALF], F32, tag="zs", name="zs")
            nc.scalar.activation(zr[w, :], accb[w, :], LN, bias=0.0, scale=1.0)
            nc.vector.tensor_copy(zh[w, :], zr[w, :])          # hi = bf16(lnZ)
            nc.vector.tensor_sub(zs[w, :], zr[w, :], zh[w, :])  # lo
            nc.vector.tensor_scalar_mul(zr[w, :], zh[w, :], msk[w, 0:1])
            nc.vector.tensor_scalar_mul(zs[w, :], zs[w, :], imsk[w, 0:1])
            nc.vector.tensor_add(zr[w, :], zr[w, :], zs[w, :])
            nc.vector.tensor_scalar_mul(qtB[w, l0:l0 + LHALF], zr[w, :], -1.0)
            # copy B (rows 0..64) to SBUF; accumulator slot then reusable
            b_sb = eppool.tile([D + 1, LHALF], F32, tag="b_sb%d" % lh,
                               name="b_sb")
            nc.vector.tensor_copy(b_sb, accb[0:D + 1, :])
            b_sbs[lh] = b_sb

        for lh in range(NLH):
            l0 = lh * LHALF
            acca = ps_acc.tile([DV, LHALF], F32, tag="acc", name="acca")
            # round B: p5 = exp(5(s - m2)); accumulate A = P5 @ [V|1s]
            for st in range(ST):
                sb = slice(st * 128, (st + 1) * 128)
                sc = ps_sc.tile([128, LHALF], F32, tag="sc", name="scB")
                for c in range(NCH):
                    cs = slice(c * 512, (c + 1) * 512)
                    gs = slice(l0 + c * 512, l0 + (c + 1) * 512)
                    nc.tensor.matmul(sc[:, cs], lhsT=ktA[:, sb],
                                     rhs=qtA[:, gs], start=True, stop=False)
                    nc.tensor.matmul(sc[:, cs], lhsT=ktB[:, sb],
                                     rhs=qtB[:, gs], start=False, stop=True)
                p5 = ppool.tile([128, LHALF], BF16, tag="p5", name="p5")
                nc.scalar.activation(p5, sc, EXP, bias=0.0, scale=FACTOR)
                for c in range(NCH):
                    cs = slice(c * 512, (c + 1) * 512)
                    nc.tensor.matmul(acca[:, cs], lhsT=vts[st], rhs=p5[:, cs],
                                     start=(st == 0), stop=(st == ST - 1))
            a_sb = eppool.tile([D + 1, LHALF], F32, tag="a_sb%d" % lh,
                               name="a_sb")
            nc.vector.tensor_copy(a_sb, acca[0:D + 1, :])
            a_sbs[lh] = a_sb

        # ---- epilogue: out = (A + B/Z_A)[:, :D] / (A + B/Z_A)[:, D] ----
        for lh in range(NLH):
            l0 = lh * LHALF
            a_sb, b_sb = a_sbs[lh], b_sbs[lh]
            for ch in range(LHALF // 128):
                at_ps = ps_sc.tile([128, D + 1], F32, tag="sc", name="at_ps")
                bt_ps = ps_sc.tile([128, D + 1], F32, tag="sc", name="bt_ps")
                nc.tensor.transpose(at_ps, a_sb[:, ch * 128:(ch + 1) * 128],
                                    ident65)
                nc.tensor.transpose(bt_ps, b_sb[:, ch * 128:(ch + 1) * 128],
                                    ident65)
                z = zpool.tile([128, 4], F32, tag="z", name="z")
                nc.vector.reciprocal(z[:, 0:1], bt_ps[:, D:D + 1])  # 1/Z_A
                n65 = opool.tile([128, D + 1], F32, tag="n65", name="n65")
                nc.vector.tensor_scalar_mul(n65, bt_ps, z[:, 0:1])
                nc.vector.tensor_add(n65, n65, at_ps)
                nc.vector.reciprocal(z[:, 1:2], n65[:, D:D + 1])    # 1/den
                ot = opool.tile([128, D], F32, tag="ot", name="ot")
                nc.vector.tensor_scalar_mul(ot, n65[:, 0:D], z[:, 1:2])
                lrow = l0 + ch * 128
                nc.gpsimd.dma_start(out=outp[bh, lrow:lrow + 128, :], in_=ot)


_CACHE = {}


def _build():
    if "nc" in _CACHE:
        return _CACHE["nc"]
    nc = bass.Bass()
    qa = nc.declare_dram_parameter("qa", [NP, L, 128], BF16, isOutput=False)
    qb = nc.declare_dram_parameter("qb", [NP, L, EB], BF16, isOutput=False)
    ka = nc.declare_dram_parameter("ka", [NP, S, 128], BF16, isOutput=False)
    kb = nc.declare_dram_parameter("kb", [NP, S, EB], BF16, isOutput=False)
    va = nc.declare_dram_parameter("va", [NP, S, DV], BF16, isOutput=False)
    outp = nc.declare_dram_parameter("out", [NP, L, D], F32, isOutput=True)
    with tile.TileContext(nc) as tc:
        with ExitStack() as ctx:
            _emit(ctx, tc, qa[:], qb[:], ka[:], kb[:], va[:], outp[:])
    _CACHE["nc"] = nc
    return nc


def _prep_inputs(queries, keys, values):
    bf = ml_dtypes.bfloat16
    q = np.ascontiguousarray(np.asarray(queries, np.float32).transpose(0, 2, 1, 3)
                             ).reshape(B * H, L, E)
    k = np.ascontiguousarray(np.asarray(keys, np.float32).transpose(0, 2, 1, 3)
                             ).reshape(B * H, S, E)
    v = np.ascontiguousarray(np.asarray(values, np.float32).transpose(0, 2, 1, 3)
                             ).reshape(B * H, S, D)
    qhi = q.astype(bf)
    qlo = (q - qhi.astype(np.float32)).astype(bf)
    khi = k.astype(bf)
    klo = (k - khi.astype(np.float32)).astype(bf)
    m1 = (M_COEF * np.sqrt((q.astype(np.float64) ** 2).sum(-1)) + M_MARGIN
          ).astype(np.float32)  # [BH, L]
    zero_l = np.zeros((B * H, L, 1), bf)
    one_s = np.ones((B * H, S, 1), bf)
    qa = np.concatenate([qhi, qlo], axis=-1)                        # [.,L,128]
    qb = np.concatenate([qhi, zero_l, zero_l, (-m1[..., None]).astype(bf)],
                        axis=-1)                                    # [.,L,67]
    ka = np.concatenate([khi, khi], axis=-1)                        # [.,S,128]
    kb = np.concatenate([klo, one_s, one_s, one_s], axis=-1)        # [.,S,67]
    va = np.concatenate([v.astype(bf), one_s, one_s], axis=-1)      # [.,S,66]
    in_maps = []
    for c in range(NCORES):
        sl = slice(c * NP, (c + 1) * NP)
        in_maps.append({
            "qa": np.ascontiguousarray(qa[sl]),
            "qb": np.ascontiguousarray(qb[sl]),
            "ka": np.ascontiguousarray(ka[sl]),
            "kb": np.ascontiguousarray(kb[sl]),
            "va": np.ascontiguousarray(va[sl]),
        })
    return in_maps


def _gather(results):
    outs = np.stack([results[c]["out"] for c in range(NCORES)])  # [8, NP, L, D]
    out = outs.reshape(B, H, L, D).transpose(0, 2, 1, 3)
    return np.ascontiguousarray(out)


def run_sharded(queries, keys, values, **kw):
    """Run on the 8 neuron cores; returns (full_output, BassKernelResults)."""
    nc = _build()
    in_maps = _prep_inputs(queries, keys, values)
    res = run_bass_kernel_spmd(nc, in_maps, list(range(NCORES)), **kw)
    return _gather(res.results), res


def kernel(queries, keys, values):
    out, _ = run_sharded(queries, keys, values)
    return out



# revision 9
# speedup vs baseline: 1.2633x; 1.2633x over previous
"""Trainium2 Bass kernel for DynamicSparseAttention (v2 — single-score-pass).

Reference computation (per batch b, head h):
    scores  = Q @ K^T                      [L, S]
    dense   = softmax(scores, axis=-1)
    routing = dense ** 5
    combined = (routing + dense) * 0.5
    sparse  = combined / sum(combined, -1, keepdims=True)
    out     = sparse @ V                   [L, D]

Math.  With p = exp(s - m1) (m1 a loose per-row upper bound on s), Z = sum_s p,
q = p/Z (the true softmax), and w = q^5 + q:
    out = (W @ V) / (W @ 1) = (A5 + B.zinv) / (W5 + Z.zinv)
where B = P@[V|1] (dense sums, Z in its ones row), A5 = Q5@[V|1] (W5 in its
ones row).  Scores are computed ONCE (bf16 hi/lo split for fp32 accuracy):
    s = khi.(qhi+qlo) + klo.qhi - m1     (two accumulating bf16 matmuls)
q^5 per s-tile comes from one of two per-tile paths (mix balances engines):
  E-path: p5 = exp(5(s - lnZ)) on the scalar engine (s parked in a DRAM
          scratch during round A, lnZ broadcast subtracted on DVE).
  P-path: q = p*zinv; q2 = q^2; q4 = q2^2; q5 = q4*q   (bf16 DVE muls).
Both produce the same quantity; numerics validated to ~3.5e-3 max rel err.

Layout: scores transposed [s partitions, l free]; V' = [v|1] stationary.
Output written transposed [D, L] per pair; host gathers/transposes.

Software pipeline: units u = (pair, l-half); while unit u streams its score
matmuls + exp + PV-B (phase A), unit u-1 runs round B (powering / exp5 +
PV-A5) and unit u-2 finishes its epilogue.  PSUM: 2 score bufs (4 banks) +
one accB (2) + one accA5 (2) = 8 banks.

Sharding: B*H = 32 (b,h) pairs, 4 per core across 8 cores, no cross-core
communication.  kernel() takes full inputs and returns the full output.
"""

import os
import sys
import numpy as np

for _p in ("/opt/trn_rl_repo",):
    if os.path.isdir(_p) and _p not in sys.path:
        sys.path.insert(0, _p)

from contextlib import ExitStack

import json as _json

import ml_dtypes

import concourse.bass as bass
import concourse.mybir as mybir
import concourse.tile as tile
import concourse.bass2jax as _bass2jax
import concourse.bass_utils as _bass_utils
from concourse.bass_utils import run_bass_kernel_spmd

# ---------------------------------------------------------------------------
# Workaround: this container's walrus build rejects instructions carrying
# more than one sync wait.  Rewrite the BIR JSON before compilation: excess
# waits are hoisted onto freshly inserted same-engine NoOp instructions
# placed immediately before the instruction, one wait each.
# ---------------------------------------------------------------------------

_MAX_WAITS = 1


def _split_waits_in_bir(bir_json: bytes) -> bytes:
    bir = _json.loads(bir_json)
    n_new = [0]

    def fix_block(bb):
        out = []
        for inst in bb["instructions"]:
            si = inst.get("sync_info") or {}
            waits = si.get("on_wait") or []
            if len(waits) > _MAX_WAITS:
                excess, keep = waits[:-_MAX_WAITS], waits[-_MAX_WAITS:]
                for w in excess:
                    n_new[0] += 1
                    out.append({
                        "debug": inst.get("debug", 0),
                        "engine": inst["engine"],
                        "ins": [],
                        "name": "I-wsplit-%d" % n_new[0],
                        "opcode": "NoOp",
                        "outs": [],
                        "sync_info": {"on_update": [], "on_wait": [w]},
                    })
                si["on_wait"] = keep
            out.append(inst)
        bb["instructions"] = out

    for fn in bir["functions"]:
        for bb in fn["blocks"]:
            fix_block(bb)
    return _json.dumps(bir).encode()


_orig_compile_bir_kernel = _bass_utils.compile_bir_kernel


def _patched_compile_bir_kernel(bir_json, tmpdir, neff_name="file.neff"):
    return _orig_compile_bir_kernel(
        _split_waits_in_bir(bir_json), tmpdir, neff_name=neff_name
    )


_bass_utils.compile_bir_kernel = _patched_compile_bir_kernel
_bass2jax.compile_bir_kernel = _patched_compile_bir_kernel

# ---------------------------------------------------------------------------
# Problem constants / tuning knobs
# ---------------------------------------------------------------------------

B, L, S, H, E, D = 2, 2048, 2048, 16, 64, 64
NCORES = 8
NP = (B * H) // NCORES   # pairs per core = 4
LHALF = 1024             # l columns per unit
NLH = L // LHALF         # 2
NU = NP * NLH            # 8 units per core
ST = S // 128            # 16 s-tiles
CHW = 512                # matmul chunk width (one PSUM bank of fp32)
NCH = LHALF // CHW       # 2
DV = D + 1               # 65: [v | 1]
KB = E + 1               # 65: [klo | 1] / [qhi | -m1]
FACTOR = 5.0

# E-path s-tiles (scalar-engine exp5); the rest use DVE powering.
E_LIST = (3, 6, 9, 12, 15)
GP_Q2_SET = frozenset({1, 4, 8, 11, 14})  # P-tiles whose q^2 runs on gpsimd
E_SET = frozenset(E_LIST)
NE = len(E_LIST)
B_DELAY = 4              # iters between A(u) and interleaved B(u-1) tile ops
PVB_LEAD = 2             # PV-B(u, st) emitted at iter st+PVB_LEAD
PVA_LEAD = 7             # PV-A5(u-1, st) emitted at iter st+PVA_LEAD
NIT = ST + PVA_LEAD      # inner iterations per unit slot
_EPI_AT = {0: 0, 2: 1, 4: 2, 7: 3, 10: 4}  # iter -> epilogue step

F32 = mybir.dt.float32
BF16 = mybir.dt.bfloat16
EXP = mybir.ActivationFunctionType.Exp
LN = mybir.ActivationFunctionType.Ln

M_COEF = float(np.sqrt(2.0 * np.log(S)))
M_MARGIN = 25.0


def _e_idx(st):
    return E_LIST.index(st)


def _emit(ctx: ExitStack, tc: tile.TileContext, qta, qtb, kta, ktb, va, scr,
          zscr, zscrb, outp):
    nc = tc.nc

    oper = ctx.enter_context(tc.tile_pool(name="oper", bufs=2))
    vpool = ctx.enter_context(tc.tile_pool(name="vp", bufs=2))
    ppool = ctx.enter_context(tc.tile_pool(name="pp", bufs=2))
    spool = ctx.enter_context(tc.tile_pool(name="slp", bufs=4))
    qpool = ctx.enter_context(tc.tile_pool(name="qp", bufs=2))
    zpool = ctx.enter_context(tc.tile_pool(name="zp", bufs=1))
    epool = ctx.enter_context(tc.tile_pool(name="ep", bufs=1))

    ps_sc = ctx.enter_context(tc.tile_pool(name="ps_sc", bufs=2, space="PSUM"))
    ps_b = ctx.enter_context(tc.tile_pool(name="ps_b", bufs=1, space="PSUM"))
    ps_a = ctx.enter_context(tc.tile_pool(name="ps_a", bufs=1, space="PSUM"))

    opers = {}   # pair -> dict of operand tiles
    U = {}       # unit -> dict of state

    def load_pair(pair):
        qta_t = oper.tile([128, L], BF16, tag="qta", name="qta_t")
        nc.gpsimd.dma_start(out=qta_t, in_=qta[pair])
        qtb_t = oper.tile([KB, L], BF16, tag="qtb", name="qtb_t")
        nc.gpsimd.dma_start(out=qtb_t, in_=qtb[pair])
        kta_t = oper.tile([128, S], BF16, tag="kta", name="kta_t")
        nc.gpsimd.dma_start(out=kta_t, in_=kta[pair])
        ktb_t = oper.tile([KB, S], BF16, tag="ktb", name="ktb_t")
        nc.gpsimd.dma_start(out=ktb_t, in_=ktb[pair])
        vts = []
        for t in range(ST):
            vt = vpool.tile([128, DV], BF16, tag=f"v{t}", name=f"vt{t}")
            nc.gpsimd.dma_start(out=vt, in_=va[pair, t * 128:(t + 1) * 128, :])
            vts.append(vt)
        opers[pair] = dict(qta=qta_t, qtb=qtb_t, kta=kta_t, ktb=ktb_t,
                           vts=vts)

    def emit_A_mms(u, st):
        """Score matmuls + exp for (u, st); parks s to DRAM for E-tiles."""
        pair, lh = divmod(u, NLH)
        op = opers[pair]
        l0 = lh * LHALF
        sc = ps_sc.tile([128, LHALF], F32, tag="sc", name="sc")
        if st in E_SET:
            p = ppool.tile([128, LHALF], BF16, tag="pE", bufs=PVB_LEAD + 2,
                           name="pE")
        else:
            p = ppool.tile([128, LHALF], BF16, tag=f"pP{st}", bufs=2,
                           name="pP")
        ss = slice(st * 128, (st + 1) * 128)
        for c in range(NCH):
            cs = slice(c * CHW, (c + 1) * CHW)
            gs = slice(l0 + c * CHW, l0 + (c + 1) * CHW)
            nc.tensor.matmul(sc[:, cs], lhsT=op["kta"][:, ss],
                             rhs=op["qta"][:, gs], start=True, stop=False)
            nc.tensor.matmul(sc[:, cs], lhsT=op["ktb"][:, ss],
                             rhs=op["qtb"][:, gs], start=False, stop=True)
        nc.scalar.activation(p, sc, EXP, bias=0.0, scale=1.0)
        if st in E_SET:
            # bounce s through SBUF (gpsimd copy), park in DRAM scratch
            s_tmp = spool.tile([128, LHALF], F32, tag="stmp", bufs=2,
                               name="s_tmp")
            nc.scalar.activation(s_tmp, sc,
                                 mybir.ActivationFunctionType.Copy,
                                 bias=0.0, scale=1.0)
            nc.sync.dma_start(out=scr[u % 2, _e_idx(st)], in_=s_tmp)
        U[u]["p"][st] = p

    def emit_A_pvb(u, st):
        pair, _lh = divmod(u, NLH)
        if st == 0:
            U[u]["accB"] = ps_b.tile([DV, LHALF], F32, tag="accB",
                                     name="accB")
        accB = U[u]["accB"]
        p = U[u]["p"][st]
        vt = opers[pair]["vts"][st]
        for c in range(NCH):
            cs = slice(c * CHW, (c + 1) * CHW)
            nc.tensor.matmul(accB[:, cs], lhsT=vt, rhs=p[:, cs],
                             start=(st == 0), stop=(st == ST - 1))

    def _bcast_land(out_tile, row_ap):
        """DMA-land a parked DRAM row replicated across partitions."""
        nparts = out_tile.shape[0]
        free = row_ap.ap[-1][1]
        bap = bass.AP(row_ap.tensor, row_ap.offset, [[0, nparts], [1, free]])
        nc.sync.dma_start(out=out_tile, in_=bap)

    def emit_mid(u):
        """After PV-B(u,15): zinv/lnZ rows, broadcasts, accB copy-out."""
        accB = U[u]["accB"]
        b_sb = epool.tile([DV, LHALF], F32, tag="bsb", bufs=2, name="b_sb")
        nc.scalar.activation(b_sb, accB, mybir.ActivationFunctionType.Copy,
                             bias=0.0, scale=1.0)   # frees accB (bufs=1)
        zr = zpool.tile([1, LHALF], F32, tag="zr", name="zr")
        nc.vector.reciprocal(zr, accB[D:DV, :])
        zr16 = zpool.tile([1, LHALF], BF16, tag="zr16", name="zr16")
        nc.vector.tensor_copy(zr16, zr)
        lnz = zpool.tile([1, LHALF], F32, tag="lnz", name="lnz")
        nc.scalar.activation(lnz, accB[D:DV, :], LN, bias=0.0, scale=1.0)
        # park rows to DRAM, land them partition-replicated (stride-0 reads)
        par = u % 2
        nc.sync.dma_start(out=zscrb[par, 0:1, :], in_=zr16)
        nc.sync.dma_start(out=zscr[par, 0:1, :], in_=zr)
        nc.sync.dma_start(out=zscr[par, 1:2, :], in_=lnz)
        zb16 = zpool.tile([128, LHALF], BF16, tag="zb16", name="zb16")
        _bcast_land(zb16, zscrb[par, 0:1, :])
        m2b = zpool.tile([128, LHALF], F32, tag="m2b", name="m2b")
        _bcast_land(m2b, zscr[par, 1:2, :])
        zi32b = zpool.tile([DV, LHALF], F32, tag="zi32b", bufs=2,
                           name="zi32b")
        _bcast_land(zi32b, zscr[par, 0:1, :])
        U[u].update(b_sb=b_sb, zb16=zb16, m2b=m2b, zi32b=zi32b)

    def emit_landing(u, st):
        """Bring parked s back from DRAM shortly before its round-B sub."""
        sl = spool.tile([128, LHALF], F32, tag="sland", bufs=3, name="sland")
        nc.sync.dma_start(out=sl, in_=scr[u % 2, _e_idx(st)])
        U[u]["sland"][st] = sl

    def emit_B_tile(u, st):
        """Round-B rhs for (u, st): p5 via exp5 (E) or q^5 via powering."""
        if st in E_SET:
            sp = qpool.tile([128, LHALF], F32, tag="sp", bufs=2, name="sp")
            nc.gpsimd.tensor_sub(sp, U[u]["sland"][st], U[u]["m2b"])
            r5 = qpool.tile([128, LHALF], BF16, tag="p5", bufs=PVA_LEAD - B_DELAY + 3,
                            name="p5")
            nc.scalar.activation(r5, sp, EXP, bias=0.0, scale=FACTOR)
        else:
            p = U[u]["p"][st]
            q = qpool.tile([128, LHALF], BF16, tag="q", bufs=2, name="q")
            nc.vector.tensor_mul(q, p, U[u]["zb16"])
            q2 = qpool.tile([128, LHALF], BF16, tag="q2", bufs=2, name="q2")
            if st in GP_Q2_SET:
                nc.gpsimd.tensor_mul(q2, q, q)
            else:
                nc.vector.tensor_mul(q2, q, q)
            q4 = qpool.tile([128, LHALF], BF16, tag="q4", bufs=2, name="q4")
            nc.vector.tensor_mul(q4, q2, q2)
            r5 = qpool.tile([128, LHALF], BF16, tag="q5", bufs=PVA_LEAD - B_DELAY + 3,
                            name="q5")
            nc.vector.tensor_mul(r5, q4, q)
        U[u]["r5"][st] = r5

    def emit_B_pva5(u, st):
        pair, _lh = divmod(u, NLH)
        if st == 0:
            U[u]["accA"] = ps_a.tile([DV, LHALF], F32, tag="accA",
                                     name="accA")
        accA = U[u]["accA"]
        r5 = U[u]["r5"][st]
        vt = opers[pair]["vts"][st]
        for c in range(NCH):
            cs = slice(c * CHW, (c + 1) * CHW)
            nc.tensor.matmul(accA[:, cs], lhsT=vt, rhs=r5[:, cs],
                             start=(st == 0), stop=(st == ST - 1))

    def emit_epi(u, step):
        pair, lh = divmod(u, NLH)
        l0 = lh * LHALF
        st8 = U[u]
        if step == 0:
            n = epool.tile([DV, LHALF], F32, tag="n", bufs=1, name="n")
            nc.gpsimd.tensor_mul(n, st8["b_sb"], st8["zi32b"])
            st8["n"] = n
        elif step == 1:
            n2 = epool.tile([DV, LHALF], F32, tag="n2", bufs=1, name="n2")
            nc.vector.tensor_add(n2, st8["n"], st8["accA"])  # frees accA
            st8["n2"] = n2
        elif step == 2:
            dr = epool.tile([1, LHALF], F32, tag="dr", bufs=1, name="dr")
            nc.vector.reciprocal(dr, st8["n2"][D:DV, :])
            nc.sync.dma_start(out=zscr[u % 2, 2:3, :], in_=dr)
            st8["dr"] = dr
        elif step == 3:
            dinvb = epool.tile([D, LHALF], F32, tag="dinvb", bufs=1,
                               name="dinvb")
            _bcast_land(dinvb, zscr[u % 2, 2:3, :])
            st8["dinvb"] = dinvb
        elif step == 4:
            ot = epool.tile([D, LHALF], F32, tag="ot", bufs=1, name="ot")
            nc.vector.tensor_mul(ot, st8["n2"][0:D, :], st8["dinvb"])
            nc.gpsimd.dma_start(out=outp[pair, :, l0:l0 + LHALF], in_=ot)

    # ---- main pipeline ---------------------------------------------------
    load_pair(0)
    for u in range(NU + 1):
        if u < NU:
            U[u] = dict(p={}, r5={}, sland={})
            pair, lh = divmod(u, NLH)
        for it in range(NIT):
            if u < NU and it < ST:
                emit_A_mms(u, it)
            if u < NU and PVB_LEAD <= it < ST + PVB_LEAD:
                emit_A_pvb(u, it - PVB_LEAD)
            if u >= 1 and (it + 1) in E_SET and it + 1 < ST:
                emit_landing(u - 1, it + 1)
            if u >= 1 and B_DELAY <= it < ST + B_DELAY:
                emit_B_tile(u - 1, it - B_DELAY)
            if u >= 1 and PVA_LEAD <= it:
                emit_B_pva5(u - 1, it - PVA_LEAD)
            if u >= 2 and it in _EPI_AT:
                emit_epi(u - 2, _EPI_AT[it])
            if u < NU and lh == 0 and it == 10 and pair + 1 < NP:
                load_pair(pair + 1)
        if u < NU:
            emit_mid(u)
            if 0 in E_SET or 1 in E_SET or 2 in E_SET:
                for st in E_LIST:
                    if st <= B_DELAY:
                        emit_landing(u, st)
    # epilogue of the last unit
    for step in range(5):
        emit_epi(NU - 1, step)


_CACHE = {}


def _build():
    if "nc" in _CACHE:
        return _CACHE["nc"]
    nc = bass.Bass()
    qta = nc.declare_dram_parameter("qta", [NP, 128, L], BF16, isOutput=False)
    qtb = nc.declare_dram_parameter("qtb", [NP, KB, L], BF16, isOutput=False)
    kta = nc.declare_dram_parameter("kta", [NP, 128, S], BF16, isOutput=False)
    ktb = nc.declare_dram_parameter("ktb", [NP, KB, S], BF16, isOutput=False)
    va = nc.declare_dram_parameter("va", [NP, S, DV], BF16, isOutput=False)
    outp = nc.declare_dram_parameter("out", [NP, D, L], F32, isOutput=True)
    scr = nc.dram_tensor("sscr", [2, NE, 128, LHALF], F32, kind="Internal")
    zscr = nc.dram_tensor("zscr", [2, 3, LHALF], F32, kind="Internal")
    zscrb = nc.dram_tensor("zscrb", [2, 1, LHALF], BF16, kind="Internal")
    with tile.TileContext(nc) as tc:
        with ExitStack() as ctx:
            _emit(ctx, tc, qta[:], qtb[:], kta[:], ktb[:], va[:], scr[:],
                  zscr[:], zscrb[:], outp[:])
    _CACHE["nc"] = nc
    return nc


def _prep_inputs(queries, keys, values):
    bf = ml_dtypes.bfloat16
    q = np.ascontiguousarray(
        np.asarray(queries, np.float32).transpose(0, 2, 1, 3)
    ).reshape(B * H, L, E)
    k = np.ascontiguousarray(
        np.asarray(keys, np.float32).transpose(0, 2, 1, 3)
    ).reshape(B * H, S, E)
    v = np.ascontiguousarray(
        np.asarray(values, np.float32).transpose(0, 2, 1, 3)
    ).reshape(B * H, S, D)
    qhi = q.astype(bf)
    qlo = (q - qhi.astype(np.float32)).astype(bf)
    khi = k.astype(bf)
    klo = (k - khi.astype(np.float32)).astype(bf)
    m1 = (M_COEF * np.sqrt((q.astype(np.float64) ** 2).sum(-1)) + M_MARGIN
          ).astype(np.float32)  # [BH, L]
    one_s = np.ones((B * H, 1, S), bf)
    qta = np.concatenate([qhi.transpose(0, 2, 1),
                          qlo.transpose(0, 2, 1)], axis=1)          # [.,128,L]
    qtb = np.concatenate([qhi.transpose(0, 2, 1),
                          (-m1[:, None, :]).astype(bf)], axis=1)    # [.,65,L]
    kta = np.concatenate([khi.transpose(0, 2, 1),
                          khi.transpose(0, 2, 1)], axis=1)          # [.,128,S]
    ktb = np.concatenate([klo.transpose(0, 2, 1), one_s], axis=1)   # [.,65,S]
    va = np.concatenate([v.astype(bf), np.ones((B * H, S, 1), bf)],
                        axis=-1)                                    # [.,S,65]
    in_maps = []
    for c in range(NCORES):
        sl = slice(c * NP, (c + 1) * NP)
        in_maps.append({
            "qta": np.ascontiguousarray(qta[sl]),
            "qtb": np.ascontiguousarray(qtb[sl]),
            "kta": np.ascontiguousarray(kta[sl]),
            "ktb": np.ascontiguousarray(ktb[sl]),
            "va": np.ascontiguousarray(va[sl]),
        })
    return in_maps


def _gather(results):
    outs = np.stack([results[c]["out"] for c in range(NCORES)])  # [8,NP,D,L]
    out = outs.reshape(B, H, D, L).transpose(0, 3, 1, 2)  # [B, L, H, D]
    return np.ascontiguousarray(out)


def run_sharded(queries, keys, values, **kw):
    """Run on the 8 neuron cores; returns (full_output, BassKernelResults)."""
    nc = _build()
    in_maps = _prep_inputs(queries, keys, values)
    res = run_bass_kernel_spmd(nc, in_maps, list(range(NCORES)), **kw)
    return _gather(res.results), res


def kernel(queries, keys, values):
    out, _ = run_sharded(queries, keys, values)
    return out


# revision 11
# speedup vs baseline: 1.4291x; 1.1313x over previous
"""Trainium2 Bass kernel for DynamicSparseAttention (v2 — single-score-pass).

Reference computation (per batch b, head h):
    scores  = Q @ K^T                      [L, S]
    dense   = softmax(scores, axis=-1)
    routing = dense ** 5
    combined = (routing + dense) * 0.5
    sparse  = combined / sum(combined, -1, keepdims=True)
    out     = sparse @ V                   [L, D]

Math.  With p = exp(s - m1) (m1 a loose per-row upper bound on s), Z = sum_s p,
q = p/Z (the true softmax), and w = q^5 + q:
    out = (W @ V) / (W @ 1) = (A5 + B.zinv) / (W5 + Z.zinv)
where B = P@[V|1] (dense sums, Z in its ones row), A5 = Q5@[V|1] (W5 in its
ones row).  Scores are computed ONCE (bf16 hi/lo split for fp32 accuracy):
    s = khi.(qhi+qlo) + klo.qhi - m1     (two accumulating bf16 matmuls)
q^5 per s-tile comes from one of two per-tile paths (mix balances engines):
  E-path: p5 = exp(5(s - lnZ)) on the scalar engine (s parked in a DRAM
          scratch during round A, lnZ broadcast subtracted on DVE).
  P-path: q = p*zinv; q2 = q^2; q4 = q2^2; q5 = q4*q   (bf16 DVE muls).
Both produce the same quantity; numerics validated to ~3.5e-3 max rel err.

Layout: scores transposed [s partitions, l free]; V' = [v|1] stationary.
Output written transposed [D, L] per pair; host gathers/transposes.

Software pipeline: units u = (pair, l-half); while unit u streams its score
matmuls + exp + PV-B (phase A), unit u-1 runs round B (powering / exp5 +
PV-A5) and unit u-2 finishes its epilogue.  PSUM: 2 score bufs (4 banks) +
one accB (2) + one accA5 (2) = 8 banks.

Sharding: B*H = 32 (b,h) pairs, 4 per core across 8 cores, no cross-core
communication.  kernel() takes full inputs and returns the full output.
"""

import os
import sys
import numpy as np

for _p in ("/opt/trn_rl_repo",):
    if os.path.isdir(_p) and _p not in sys.path:
        sys.path.insert(0, _p)

from contextlib import ExitStack

import json as _json

import ml_dtypes

import concourse.bass as bass
import concourse.mybir as mybir
import concourse.tile as tile
import concourse.bass2jax as _bass2jax
import concourse.bass_utils as _bass_utils
from concourse.bass_utils import run_bass_kernel_spmd

# ---------------------------------------------------------------------------
# Workaround: this container's walrus build rejects instructions carrying
# more than one sync wait.  Rewrite the BIR JSON before compilation: excess
# waits are hoisted onto freshly inserted same-engine NoOp instructions
# placed immediately before the instruction, one wait each.
# ---------------------------------------------------------------------------

_MAX_WAITS = 1


def _split_waits_in_bir(bir_json: bytes) -> bytes:
    bir = _json.loads(bir_json)
    n_new = [0]

    def fix_block(bb):
        out = []
        for inst in bb["instructions"]:
            si = inst.get("sync_info") or {}
            waits = si.get("on_wait") or []
            if len(waits) > _MAX_WAITS:
                excess, keep = waits[:-_MAX_WAITS], waits[-_MAX_WAITS:]
                for w in excess:
                    n_new[0] += 1
                    out.append({
                        "debug": inst.get("debug", 0),
                        "engine": inst["engine"],
                        "ins": [],
                        "name": "I-wsplit-%d" % n_new[0],
                        "opcode": "NoOp",
                        "outs": [],
                        "sync_info": {"on_update": [], "on_wait": [w]},
                    })
                si["on_wait"] = keep
            out.append(inst)
        bb["instructions"] = out

    for fn in bir["functions"]:
        for bb in fn["blocks"]:
            fix_block(bb)
    return _json.dumps(bir).encode()


_orig_compile_bir_kernel = _bass_utils.compile_bir_kernel


def _patched_compile_bir_kernel(bir_json, tmpdir, neff_name="file.neff"):
    return _orig_compile_bir_kernel(
        _split_waits_in_bir(bir_json), tmpdir, neff_name=neff_name
    )


_bass_utils.compile_bir_kernel = _patched_compile_bir_kernel
_bass2jax.compile_bir_kernel = _patched_compile_bir_kernel

# ---------------------------------------------------------------------------
# Problem constants / tuning knobs
# ---------------------------------------------------------------------------

B, L, S, H, E, D = 2, 2048, 2048, 16, 64, 64
NCORES = 8
NP = (B * H) // NCORES   # pairs per core = 4
LHALF = 1024             # l columns per unit
NLH = L // LHALF         # 2
NU = NP * NLH            # 8 units per core
ST = S // 128            # 16 s-tiles
CHW = 512                # matmul chunk width (one PSUM bank of fp32)
NCH = LHALF // CHW       # 2
DV = D + 1               # 65: [v | 1]
KB = E + 1               # 65: [klo | 1] / [qhi | -m1]
FACTOR = 5.0

# E-path s-tiles (scalar-engine exp5); the rest use DVE powering.
E_LIST = (3, 7, 11, 15)
GP_Q2_SET = frozenset({1, 5, 9, 13, 14})  # P-tiles whose q^2 runs on gpsimd
E_SET = frozenset(E_LIST)
NE = len(E_LIST)
B_DELAY = 4              # iters between A(u) and interleaved B(u-1) tile ops
PVB_LEAD = 2             # PV-B(u, st) emitted at iter st+PVB_LEAD
PVA_LEAD = 7             # PV-A5(u-1, st) emitted at iter st+PVA_LEAD
NIT = ST + PVA_LEAD      # inner iterations per unit slot
_EPI_AT = {0: 0, 2: 1, 4: 2, 7: 3, 10: 4}  # iter -> epilogue step

F32 = mybir.dt.float32
BF16 = mybir.dt.bfloat16
EXP = mybir.ActivationFunctionType.Exp
LN = mybir.ActivationFunctionType.Ln

M_COEF = float(np.sqrt(2.0 * np.log(S)))
M_MARGIN = 25.0


def _e_idx(st):
    return E_LIST.index(st)


def _emit(ctx: ExitStack, tc: tile.TileContext, qta, qtb, kta, ktb, va, scr,
          zscr, zscrb, outp):
    nc = tc.nc

    oper = ctx.enter_context(tc.tile_pool(name="oper", bufs=2))
    vpool = ctx.enter_context(tc.tile_pool(name="vp", bufs=2))
    ppool = ctx.enter_context(tc.tile_pool(name="pp", bufs=2))
    spool = ctx.enter_context(tc.tile_pool(name="slp", bufs=4))
    qpool = ctx.enter_context(tc.tile_pool(name="qp", bufs=2))
    zpool = ctx.enter_context(tc.tile_pool(name="zp", bufs=1))
    epool = ctx.enter_context(tc.tile_pool(name="ep", bufs=1))

    ps_sc = ctx.enter_context(tc.tile_pool(name="ps_sc", bufs=2, space="PSUM"))
    ps_b = ctx.enter_context(tc.tile_pool(name="ps_b", bufs=1, space="PSUM"))
    ps_a = ctx.enter_context(tc.tile_pool(name="ps_a", bufs=1, space="PSUM"))

    opers = {}   # pair -> dict of operand tiles
    U = {}       # unit -> dict of state

    def load_pair(pair):
        qta_t = oper.tile([128, L], BF16, tag="qta", name="qta_t")
        nc.gpsimd.dma_start(out=qta_t, in_=qta[pair])
        qtb_t = oper.tile([KB, L], BF16, tag="qtb", name="qtb_t")
        nc.gpsimd.dma_start(out=qtb_t, in_=qtb[pair])
        kta_t = oper.tile([128, S], BF16, tag="kta", name="kta_t")
        nc.gpsimd.dma_start(out=kta_t, in_=kta[pair])
        ktb_t = oper.tile([KB, S], BF16, tag="ktb", name="ktb_t")
        nc.gpsimd.dma_start(out=ktb_t, in_=ktb[pair])
        vts = []
        for t in range(ST):
            vt = vpool.tile([128, DV], BF16, tag=f"v{t}", name=f"vt{t}")
            nc.gpsimd.dma_start(out=vt, in_=va[pair, t * 128:(t + 1) * 128, :])
            vts.append(vt)
        opers[pair] = dict(qta=qta_t, qtb=qtb_t, kta=kta_t, ktb=ktb_t,
                           vts=vts)

    def emit_A_mms(u, st):
        """Score matmuls + exp for (u, st); parks s to DRAM for E-tiles."""
        pair, lh = divmod(u, NLH)
        op = opers[pair]
        l0 = lh * LHALF
        sc = ps_sc.tile([128, LHALF], F32, tag="sc", name="sc")
        if st in E_SET:
            p = ppool.tile([128, LHALF], BF16, tag="pE", bufs=PVB_LEAD + 1,
                           name="pE")
        else:
            p = ppool.tile([128, LHALF], BF16, tag=f"pP{st}", bufs=2,
                           name="pP")
        ss = slice(st * 128, (st + 1) * 128)
        for c in range(NCH):
            cs = slice(c * CHW, (c + 1) * CHW)
            gs = slice(l0 + c * CHW, l0 + (c + 1) * CHW)
            nc.tensor.matmul(sc[:, cs], lhsT=op["kta"][:, ss],
                             rhs=op["qta"][:, gs], start=True, stop=False)
            nc.tensor.matmul(sc[:, cs], lhsT=op["ktb"][:, ss],
                             rhs=op["qtb"][:, gs], start=False, stop=True)
        nc.scalar.activation(p, sc, EXP, bias=0.0, scale=1.0)
        if st in E_SET:
            # bounce s through SBUF (gpsimd copy), park in DRAM scratch
            s_tmp = spool.tile([128, LHALF], F32, tag="stmp", bufs=2,
                               name="s_tmp")
            nc.scalar.activation(s_tmp, sc,
                                 mybir.ActivationFunctionType.Copy,
                                 bias=0.0, scale=1.0)
            nc.sync.dma_start(out=scr[u % 2, _e_idx(st)], in_=s_tmp)
        U[u]["p"][st] = p

    def emit_A_pvb(u, st):
        pair, _lh = divmod(u, NLH)
        if st == 0:
            U[u]["accB"] = ps_b.tile([DV, LHALF], F32, tag="accB",
                                     name="accB")
        accB = U[u]["accB"]
        p = U[u]["p"][st]
        vt = opers[pair]["vts"][st]
        for c in range(NCH):
            cs = slice(c * CHW, (c + 1) * CHW)
            nc.tensor.matmul(accB[:, cs], lhsT=vt, rhs=p[:, cs],
                             start=(st == 0), stop=(st == ST - 1))

    def _bcast_land(out_tile, row_ap):
        """DMA-land a parked DRAM row replicated across partitions."""
        nparts = out_tile.shape[0]
        free = row_ap.ap[-1][1]
        bap = bass.AP(row_ap.tensor, row_ap.offset, [[0, nparts], [1, free]])
        nc.sync.dma_start(out=out_tile, in_=bap)

    def emit_mid(u):
        """After PV-B(u,15): Z rows via ACT ln/exp, DMA broadcasts, accB copy."""
        accB = U[u]["accB"]
        b_sb = epool.tile([DV, LHALF], F32, tag="bsb", bufs=2, name="b_sb")
        nc.scalar.activation(b_sb, accB, mybir.ActivationFunctionType.Copy,
                             bias=0.0, scale=1.0)   # frees accB (bufs=1)
        lnz = zpool.tile([1, LHALF], F32, tag="lnz", name="lnz")
        nc.scalar.activation(lnz, accB[D:DV, :], LN, bias=0.0, scale=1.0)
        zi16r = zpool.tile([1, LHALF], BF16, tag="zi16r", name="zi16r")
        nc.scalar.activation(zi16r, lnz, EXP, bias=0.0, scale=-1.0)
        zi32r = zpool.tile([1, LHALF], F32, tag="zi32r", name="zi32r")
        nc.scalar.activation(zi32r, lnz, EXP, bias=0.0, scale=-1.0)
        # park rows, land partition-replicated (stride-0 DRAM reads)
        par = u % 2
        nc.sync.dma_start(out=zscrb[par, 0:1, :], in_=zi16r)
        nc.sync.dma_start(out=zscr[par, 0:1, :], in_=zi32r)
        nc.sync.dma_start(out=zscr[par, 1:2, :], in_=lnz)
        zb16 = zpool.tile([128, LHALF], BF16, tag="zb16", name="zb16")
        _bcast_land(zb16, zscrb[par, 0:1, :])
        m2b = zpool.tile([128, LHALF], F32, tag="m2b", name="m2b")
        _bcast_land(m2b, zscr[par, 1:2, :])
        zi32b = zpool.tile([DV, LHALF], F32, tag="zi32b", bufs=2,
                           name="zi32b")
        _bcast_land(zi32b, zscr[par, 0:1, :])
        U[u].update(b_sb=b_sb, zb16=zb16, m2b=m2b, zi32b=zi32b)

    def emit_landing(u, st):
        """Bring parked s back from DRAM shortly before its round-B sub."""
        sl = spool.tile([128, LHALF], F32, tag="sland", bufs=3, name="sland")
        nc.sync.dma_start(out=sl, in_=scr[u % 2, _e_idx(st)])
        U[u]["sland"][st] = sl

    def emit_B_tile(u, st):
        """Round-B rhs for (u, st): p5 via exp5 (E) or q^5 via powering."""
        if st in E_SET:
            sp = qpool.tile([128, LHALF], F32, tag="sp", bufs=2, name="sp")
            nc.gpsimd.tensor_sub(sp, U[u]["sland"][st], U[u]["m2b"])
            r5 = qpool.tile([128, LHALF], BF16, tag="p5", bufs=PVA_LEAD - B_DELAY + 2,
                            name="p5")
            nc.scalar.activation(r5, sp, EXP, bias=0.0, scale=FACTOR)
        else:
            p = U[u]["p"][st]
            q = qpool.tile([128, LHALF], BF16, tag="q", bufs=2, name="q")
            nc.vector.tensor_mul(q, p, U[u]["zb16"])
            q2 = qpool.tile([128, LHALF], BF16, tag="q2", bufs=2, name="q2")
            if st in GP_Q2_SET:
                nc.gpsimd.tensor_mul(q2, q, q)
            else:
                nc.vector.tensor_mul(q2, q, q)
            q4 = qpool.tile([128, LHALF], BF16, tag="q4", bufs=2, name="q4")
            nc.vector.tensor_mul(q4, q2, q2)
            r5 = qpool.tile([128, LHALF], BF16, tag="q5", bufs=PVA_LEAD - B_DELAY + 2,
                            name="q5")
            nc.vector.tensor_mul(r5, q4, q)
        U[u]["r5"][st] = r5

    def emit_B_pva5(u, st):
        pair, _lh = divmod(u, NLH)
        if st == 0:
            U[u]["accA"] = ps_a.tile([DV, LHALF], F32, tag="accA",
                                     name="accA")
        accA = U[u]["accA"]
        r5 = U[u]["r5"][st]
        vt = opers[pair]["vts"][st]
        for c in range(NCH):
            cs = slice(c * CHW, (c + 1) * CHW)
            nc.tensor.matmul(accA[:, cs], lhsT=vt, rhs=r5[:, cs],
                             start=(st == 0), stop=(st == ST - 1))

    def emit_epi(u, step):
        pair, lh = divmod(u, NLH)
        l0 = lh * LHALF
        st8 = U[u]
        if step == 0:
            n = epool.tile([DV, LHALF], F32, tag="n", bufs=1, name="n")
            nc.gpsimd.tensor_mul(n, st8["b_sb"], st8["zi32b"])
            st8["n"] = n
        elif step == 1:
            n2 = epool.tile([DV, LHALF], F32, tag="n2", bufs=1, name="n2")
            nc.vector.tensor_add(n2, st8["n"], st8["accA"])  # frees accA
            st8["n2"] = n2
        elif step == 2:
            lnd = epool.tile([1, LHALF], F32, tag="lnd", bufs=1, name="lnd")
            nc.scalar.activation(lnd, st8["n2"][D:DV, :], LN, bias=0.0,
                                 scale=1.0)
            dr = epool.tile([1, LHALF], F32, tag="dr", bufs=1, name="dr")
            nc.scalar.activation(dr, lnd, EXP, bias=0.0, scale=-1.0)
            nc.sync.dma_start(out=zscr[u % 2, 2:3, :], in_=dr)
        elif step == 3:
            dinvb = epool.tile([D, LHALF], F32, tag="dinvb", bufs=1,
                               name="dinvb")
            _bcast_land(dinvb, zscr[u % 2, 2:3, :])
            st8["dinvb"] = dinvb
        elif step == 4:
            ot = epool.tile([D, LHALF], F32, tag="ot", bufs=1, name="ot")
            nc.vector.tensor_mul(ot, st8["n2"][0:D, :], st8["dinvb"])
            nc.gpsimd.dma_start(out=outp[pair, :, l0:l0 + LHALF], in_=ot)

    # ---- main pipeline ---------------------------------------------------
    load_pair(0)
    for u in range(NU + 1):
        if u < NU:
            U[u] = dict(p={}, r5={}, sland={})
            pair, lh = divmod(u, NLH)
        for it in range(NIT):
            if u < NU and it < ST:
                emit_A_mms(u, it)
            if u < NU and PVB_LEAD <= it < ST + PVB_LEAD:
                emit_A_pvb(u, it - PVB_LEAD)
            if u >= 1 and (it + 1) in E_SET and it + 1 < ST:
                emit_landing(u - 1, it + 1)
            if u >= 1 and B_DELAY <= it < ST + B_DELAY:
                emit_B_tile(u - 1, it - B_DELAY)
            if u >= 1 and PVA_LEAD <= it:
                emit_B_pva5(u - 1, it - PVA_LEAD)
            if u >= 2 and it in _EPI_AT:
                emit_epi(u - 2, _EPI_AT[it])
            if u < NU and lh == 0 and it == 10 and pair + 1 < NP:
                load_pair(pair + 1)
        if u < NU:
            emit_mid(u)
            if 0 in E_SET or 1 in E_SET or 2 in E_SET:
                for st in E_LIST:
                    if st <= B_DELAY:
                        emit_landing(u, st)
    # epilogue of the last unit
    for step in range(5):
        emit_epi(NU - 1, step)


_CACHE = {}


def _build():
    if "nc" in _CACHE:
        return _CACHE["nc"]
    nc = bass.Bass()
    qta = nc.declare_dram_parameter("qta", [NP, 128, L], BF16, isOutput=False)
    qtb = nc.declare_dram_parameter("qtb", [NP, KB, L], BF16, isOutput=False)
    kta = nc.declare_dram_parameter("kta", [NP, 128, S], BF16, isOutput=False)
    ktb = nc.declare_dram_parameter("ktb", [NP, KB, S], BF16, isOutput=False)
    va = nc.declare_dram_parameter("va", [NP, S, DV], BF16, isOutput=False)
    outp = nc.declare_dram_parameter("out", [NP, D, L], F32, isOutput=True)
    scr = nc.dram_tensor("sscr", [2, NE, 128, LHALF], F32, kind="Internal")
    zscr = nc.dram_tensor("zscr", [2, 3, LHALF], F32, kind="Internal")
    zscrb = nc.dram_tensor("zscrb", [2, 1, LHALF], BF16, kind="Internal")
    with tile.TileContext(nc) as tc:
        with ExitStack() as ctx:
            _emit(ctx, tc, qta[:], qtb[:], kta[:], ktb[:], va[:], scr[:],
                  zscr[:], zscrb[:], outp[:])
    _CACHE["nc"] = nc
    return nc


def _prep_inputs(queries, keys, values):
    bf = ml_dtypes.bfloat16
    q = np.ascontiguousarray(
        np.asarray(queries, np.float32).transpose(0, 2, 1, 3)
    ).reshape(B * H, L, E)
    k = np.ascontiguousarray(
        np.asarray(keys, np.float32).transpose(0, 2, 1, 3)
    ).reshape(B * H, S, E)
    v = np.ascontiguousarray(
        np.asarray(values, np.float32).transpose(0, 2, 1, 3)
    ).reshape(B * H, S, D)
    qhi = q.astype(bf)
    qlo = (q - qhi.astype(np.float32)).astype(bf)
    khi = k.astype(bf)
    klo = (k - khi.astype(np.float32)).astype(bf)
    m1 = (M_COEF * np.sqrt((q.astype(np.float64) ** 2).sum(-1)) + M_MARGIN
          ).astype(np.float32)  # [BH, L]
    one_s = np.ones((B * H, 1, S), bf)
    qta = np.concatenate([qhi.transpose(0, 2, 1),
                          qlo.transpose(0, 2, 1)], axis=1)          # [.,128,L]
    qtb = np.concatenate([qhi.transpose(0, 2, 1),
                          (-m1[:, None, :]).astype(bf)], axis=1)    # [.,65,L]
    kta = np.concatenate([khi.transpose(0, 2, 1),
                          khi.transpose(0, 2, 1)], axis=1)          # [.,128,S]
    ktb = np.concatenate([klo.transpose(0, 2, 1), one_s], axis=1)   # [.,65,S]
    va = np.concatenate([v.astype(bf), np.ones((B * H, S, 1), bf)],
                        axis=-1)                                    # [.,S,65]
    in_maps = []
    for c in range(NCORES):
        sl = slice(c * NP, (c + 1) * NP)
        in_maps.append({
            "qta": np.ascontiguousarray(qta[sl]),
            "qtb": np.ascontiguousarray(qtb[sl]),
            "kta": np.ascontiguousarray(kta[sl]),
            "ktb": np.ascontiguousarray(ktb[sl]),
            "va": np.ascontiguousarray(va[sl]),
        })
    return in_maps


def _gather(results):
    outs = np.stack([results[c]["out"] for c in range(NCORES)])  # [8,NP,D,L]
    out = outs.reshape(B, H, D, L).transpose(0, 3, 1, 2)  # [B, L, H, D]
    return np.ascontiguousarray(out)


def run_sharded(queries, keys, values, **kw):
    """Run on the 8 neuron cores; returns (full_output, BassKernelResults)."""
    nc = _build()
    in_maps = _prep_inputs(queries, keys, values)
    res = run_bass_kernel_spmd(nc, in_maps, list(range(NCORES)), **kw)
    return _gather(res.results), res


def kernel(queries, keys, values):
    out, _ = run_sharded(queries, keys, values)
    return out
